# revision 66
# baseline (speedup 1.0000x reference)
"""BiDAF attention-flow kernel for 8 Trainium2 NeuronCores (Bass/Tile).

Data-parallel over batch: B=32 -> 4 batches per core on 8 cores.

Math (per batch b):
  sim[i,j] = s_proj[i] + t_proj[j] + sum_d S[i,d]*wm[d]*T[j,d]
  A        = softmax_j(sim)            (row-constant s_proj cancels)
  source_target = A @ T
  w[i]     = exp(max_j sim[i,j]) ; tgt_attn = w / sum(w)
  target_source = tgt_attn @ S         (one row, broadcast over rows)
  out      = [S | source_target | S*source_target | S*target_source]

Device strategy (memory-roofline oriented; ~72us vs 101.5us baseline):
  - The S quarter of the output is the identity; it never touches the device.
    Device emits only the 3 computed quarters, in bf16 (rel-err budget 2e-2).
  - All inputs bf16: S,T in row-major (i on partitions) and d-major (for the
    sim contraction), marshalled on host.  All constants ride in one packed
    [128,1064] DMA (each dma_start costs ~600ns of sequencer issue time).
  - sim^T (j on partitions) is computed ONCE on PE; exp applied on ACT with a
    per-partition bias = t_proj[j].  Projections become per-partition columns:
    t_proj = DVE-mul + ACT Identity-accum, s_proj = DVE scalar_tensor_tensor
    accum — no PE matmuls, no affine-row gymnastics.
  - Row max over j:  max_j exp(sim^T) == exp(max_j sim) (monotone), so the
    4 exp strips are pairwise-maxed on DVE, the [128,512] result is PE-
    transposed per 128-col block, and one DVE free-axis max yields w as
    columns.  s_proj (cancelling in A) is folded in afterwards:
    w = w' * exp(s_proj).
  - softmax normalization deferred: ones-column in the row-major inputs gives
    row sums in column 400 of the A@[T|1] / w@[S|1] matmul outputs.
  - S*target_source is computed d-major (tensor_scalar on the resident S^T at
    DVE 4x rate, scalars = target_source columns extracted by K=1 matmuls)
    and stored transposed; the host transposes it back (pure marshalling).
  - Software pipeline: input DMAs issued a batch ahead (sync engine only),
    wm-scale emitted early in DVE order (it gates the next simT), and each
    batch's target_source chain (w_tail) emitted after the NEXT batch's simT
    so its cross-engine latency hides under PE work.  S*source_target on
    Pool, stf scaling on ACT, separate PSUM rings for sim vs A@T tiles.
"""

import sys

import numpy as np
import ml_dtypes

# concourse is importable via the axon sitecustomize path; fall back to /opt.
try:
    import concourse.bass as bass
except ImportError:  # pragma: no cover
    sys.path.insert(0, "/opt/trn_rl_repo")
    import concourse.bass as bass

import concourse.mybir as mybir
import concourse.tile as tile
from concourse.bass import ts
from concourse.bass_utils import run_bass_kernel_spmd


B, LS, LT, D = 32, 512, 512, 400
N_CORES = 8
BL = B // N_CORES  # batches per core
F32 = mybir.dt.float32
BF16 = mybir.dt.bfloat16
EXP = mybir.ActivationFunctionType.Exp
AX = mybir.AxisListType.X
MULT = mybir.AluOpType.mult
ADD = mybir.AluOpType.add

# d-axis layout: d = 4p + kc (p in [0,100), kc in [0,4)) — each partition
# holds 4 consecutive d rows, so the d-major input/output DMAs move 4KB
# contiguous runs (descriptor-generation is the DMA bottleneck).
KC = [100, 100, 100, 100]


def _split_multi_waits(nc: bass.Bass) -> None:
    """This walrus build encodes at most ONE sync-wait per instruction
    ("Too many sync wait commands" in setupSyncWait).  Tile's wait pass can
    attach several sem-waits to one instruction; hoist the extras onto
    same-engine NoOp carriers immediately before it (the NX sequencer
    executes the waits in order, so semantics are identical)."""
    ctr = 0
    for fn in nc.m.functions:
        for bb in fn.blocks:
            if not any(
                i.sync_info is not None and len(i.sync_info.on_wait) > 1
                for i in bb.instructions
            ):
                continue
            new_insts = []
            for inst in bb.instructions:
                si = inst.sync_info
                if si is not None and len(si.on_wait) > 1:
                    waits = list(si.on_wait)
                    for w in waits[:-1]:
                        ctr += 1
                        nop = mybir.InstNoOp(
                            name=f"splitw-{ctr}",
                            engine=inst.engine,
                            sync_info=mybir.SyncInfo(on_wait=[w], on_update=[]),
                            bass_nofuse=True,
                        )
                        nc.register_instruction(nop, overwrite=True)
                        new_insts.append(nop)
                    del si.on_wait[:-1]
                new_insts.append(inst)
            bb.instructions[:] = new_insts


def build_program() -> bass.Bass:
    nc = bass.Bass("TRN2", target_bir_lowering=False, debug=False)

    # Per-core DRAM I/O (host feeds per-core shards).
    srow_h = nc.dram_tensor("srow", [BL, LS, 404], BF16, kind="ExternalInput").ap()
    trow_h = nc.dram_tensor("trow", [BL, LT, 404], BF16, kind="ExternalInput").ap()
    st_h = nc.dram_tensor("st", [BL, D, LS], BF16, kind="ExternalInput").ap()
    tt_h = nc.dram_tensor("tt", [BL, D, LT], BF16, kind="ExternalInput").ap()
    # wm first in its own tiny DMA — it gates batch 0's wm-scale
    wmf_h = nc.dram_tensor("wmf", [128, 4], F32, kind="ExternalInput").ap()
    # packed constants: [ws(400) | wt(400) | ident(128) | ones(128)]
    sing_h = nc.dram_tensor("sing", [128, 1056], BF16, kind="ExternalInput").ap()
    out_h = nc.dram_tensor("out", [BL, LS, 800], BF16, kind="ExternalOutput").ap()
    # S*target_source quarter, stored d-major (host re-transposes)
    out2_h = nc.dram_tensor("out2", [BL, D, LS], BF16, kind="ExternalOutput").ap()

    with tile.TileContext(nc) as tc:
        with (
            tc.tile_pool(name="singles", bufs=1) as singles,
            tc.tile_pool(name="pin", bufs=4) as pin,
            tc.tile_pool(name="pet", bufs=2) as pet,
            tc.tile_pool(name="pout", bufs=2) as pout,
            tc.tile_pool(name="pps", bufs=3, space="PSUM") as pps,
            tc.tile_pool(name="ppo", bufs=2, space="PSUM") as ppo,
            tc.tile_pool(name="pmax_ps", bufs=1, space="PSUM") as pmax_ps,
            tc.tile_pool(name="psml_ps", bufs=2, space="PSUM") as psml_ps,
        ):
            wmf = singles.tile([128, 4], F32)
            nc.sync.dma_start(out=wmf[:], in_=wmf_h)
            sing = singles.tile([128, 1056], BF16)
            nc.sync.dma_start(out=sing[:], in_=sing_h)
            wsb = sing[:, 0:400]
            wtb = sing[:, 400:800]
            ident = sing[:, 800:928]
            ones = sing[0:1, 928:1056]

            state = {}

            def prologue_dma(b):
                """Issue batch b input DMAs (sync engine only, chunk-split so
                downstream per-chunk consumers start as soon as data lands).
                Order: ttt/stt first (they gate wm-scale + simT on PE)."""
                srow = pin.tile([128, 4, 404], BF16, tag="srow")
                trow = pin.tile([128, 4, 404], BF16, tag="trow")
                stt = pin.tile([128, 4, 512], BF16, tag="stt")
                ttt = pin.tile([128, 4, 512], BF16, tag="ttt")
                nc.sync.dma_start(
                    out=ttt[0:100, :, :],
                    in_=tt_h[b].rearrange("(p k) c -> p k c", k=4),
                )
                nc.sync.dma_start(
                    out=stt[0:100, :, :],
                    in_=st_h[b].rearrange("(p k) c -> p k c", k=4),
                )
                nc.sync.dma_start(
                    out=trow[:], in_=trow_h[b].rearrange("(j p) c -> p j c", p=128)
                )
                nc.sync.dma_start(
                    out=srow[:], in_=srow_h[b].rearrange("(i p) c -> p i c", p=128)
                )
                state[b] = [srow, trow, stt, ttt]

            def prologue_compute(b):
                """Batch b DVE/ACT prep: wm-scale (gates PE simT) first, then
                the projection columns."""
                srow, trow, stt, ttt = state[b][:4]
                # projections as per-partition columns on Pool (gpsimd) —
                # keeps DVE free for the max/scale chain
                tp = pout.tile([128, 4], F32, tag="tp", bufs=4)
                sp = pout.tile([128, 4], F32, tag="sp", bufs=4)
                tg = pout.tile([128, 4, D], BF16, tag="tg")
                sg = pout.tile([128, 4, D], BF16, tag="sg")
                for jc in range(4):
                    nc.vector.tensor_mul(tg[:, jc, :], trow[:, jc, 0:D], wtb[:])
                for jc in range(4):
                    nc.scalar.activation(
                        tg[:, jc, :],
                        tg[:, jc, :],
                        mybir.ActivationFunctionType.Identity,
                        accum_out=tp[:, jc : jc + 1],
                    )
                for ic in range(4):
                    nc.vector.scalar_tensor_tensor(
                        out=sg[:, ic, :],
                        in0=srow[:, ic, 0:D],
                        scalar=1.0,
                        in1=wsb[:],
                        op0=MULT,
                        op1=MULT,
                        accum_out=sp[:, ic : ic + 1],
                    )
                state[b] = [srow, trow, stt, ttt, tp, sp]

            def simT_pass(b):
                """E^T = exp(sim^T + t_proj[j]) with t_proj as ACT bias."""
                srow, trow, stt, ttt, tp, sp = state[b]
                et = pet.tile([128, 4, 512], BF16, tag="et")
                state[b].append(et)
                for jc in range(4):
                    ps = pps.tile([128, 512], F32, tag="psbig")
                    for kc in range(4):
                        p = KC[kc]
                        nc.tensor.matmul(
                            ps[:],
                            lhsT=ttt[0:p, kc, ts(jc, 128)],
                            rhs=stt[0:p, kc, :],
                            start=(kc == 0),
                            stop=(kc == 3),
                        )
                    nc.scalar.activation(
                        et[:, jc, :], ps[:], EXP, bias=tp[:, jc : jc + 1]
                    )

            def wm_scale(b):
                """Scale T^T by wm in place (DVE 4x mode).  Emitted right
                after the previous batch's pairmax so it lands early in DVE
                order — it gates the next batch's simT on PE."""
                ttt = state[b][3]
                for kc in range(4):
                    p = KC[kc]
                    nc.vector.tensor_scalar_mul(
                        ttt[0:p, kc, :], ttt[0:p, kc, :], wmf[0:p, kc : kc + 1]
                    )

            def rest_front(b):
                srow, trow, stt, ttt, tp, sp, et = state[b]

                # pairwise max of the 4 exp strips, split DVE/Pool so the two
                # halves run concurrently
                em2 = pout.tile([128, 2, 512], BF16, tag="em2")
                em = pout.tile([128, 512], BF16, tag="em")
                nc.vector.tensor_max(em2[:, 0, :], et[:, 0, :], et[:, 1, :])
                nc.vector.tensor_max(em2[:, 1, :], et[:, 2, :], et[:, 3, :])
                nc.vector.tensor_max(em[:], em2[:, 0, :], em2[:, 1, :])
                if b + 1 < BL:
                    wm_scale(b + 1)
                spe = pout.tile([128, 4], BF16, tag="spe", bufs=4)

                # epilogue matmuls + row-max transposes, interleaved so PE
                # has A@T work while DVE catches up on the max chain.
                po = {}
                # stf and sxst interleaved per row so both quarters leave in
                # ONE DMA with 1600B contiguous dst rows (descriptor-gen is
                # the DMA bottleneck, and descriptors are per contiguous run)
                sto = pout.tile([128, 4, 2 * D], BF16, tag="sto")
                stf = sto[:, :, 0:D]
                sxst = sto[:, :, D : 2 * D]
                rinv = pout.tile([128, 4], F32, tag="rinv", bufs=4)
                wraw = pout.tile([128, 4], BF16, tag="wraw", bufs=4)
                wtile = pout.tile([128, 4], BF16, tag="wtile", bufs=4)

                last = b == BL - 1

                def at_wave(ic):
                    """source_target += A@[T|1] for i-chunk ic (deferred scale)."""
                    po[ic] = ppo.tile(
                        [128, 512], F32, tag="psbig2", name=f"po{ic}"
                    )
                    for jc in range(4):
                        nc.tensor.matmul(
                            po[ic][:, 0:401],
                            lhsT=et[:, jc, ts(ic, 128)],
                            rhs=trow[:, jc, 0:401],
                            start=(jc == 0),
                            stop=(jc == 3),
                        )
                    nc.vector.reciprocal(rinv[:, ic : ic + 1], po[ic][:, 400:401])
                    if ic == 1:
                        # free po[1]'s PSUM bank early (po[3] reuses it): DVE
                        # scales it right after the reciprocal instead of
                        # queueing behind ACT's exp/stf backlog
                        nc.vector.tensor_scalar_mul(
                            stf[:, ic, :], po[ic][:, 0:D], rinv[:, ic : ic + 1]
                        )
                    else:
                        nc.scalar.mul(
                            stf[:, ic, :], po[ic][:, 0:D], rinv[:, ic : ic + 1]
                        )
                    nc.gpsimd.tensor_mul(
                        sxst[:, ic, :], srow[:, ic, 0:D], stf[:, ic, :]
                    )
                    if last:
                        nc.scalar.dma_start(
                            out=out_h[b, ts(ic, 128), :], in_=sto[:, ic, :]
                        )

                def max_chain():
                    # row-max via PE transpose of em blocks + one merged DVE max
                    tps = pmax_ps.tile([128, 4, 128], BF16, tag="maxps")
                    for ic in range(4):
                        nc.tensor.transpose(
                            tps[:, ic, :], em[:, ts(ic, 128)], ident[:]
                        )
                    for ic in range(4):
                        nc.vector.reduce_max(
                            wraw[:, ic : ic + 1], tps[:, ic, :], axis=AX
                        )
                    # exp(s_proj) for the w fixup — emitted here so the stf
                    # chain ahead of it on ACT isn't delayed
                    nc.scalar.activation(spe[:], sp[:], EXP)
                    nc.vector.tensor_mul(wtile[:], wraw[:], spe[:])

                at_wave(0)
                at_wave(1)
                max_chain()
                at_wave(2)
                at_wave(3)
                if not last:
                    # one DMA for both quarters: 1600B contiguous dst rows
                    nc.gpsimd.dma_start(
                        out=out_h[b].rearrange("(i p) c -> p i c", p=128),
                        in_=sto[:],
                    )
                state[b].append(wtile)

            def w_tail(b):
                """The target_source chain for batch b.  Emitted after batch
                b+1's simT so its cross-engine latency hides under PE work."""
                srow, trow, stt, ttt, tp, sp, et, wtile = state[b]
                last = b == BL - 1

                # target_source = (w @ [S|1]) / sum(w)
                ps_ts = psml_ps.tile([1, 402], F32, tag="pssml", bufs=1, name="ps_ts")
                for ic in range(4):
                    nc.tensor.matmul(
                        ps_ts[0:1, 0:401],
                        lhsT=wtile[:, ic : ic + 1],
                        rhs=srow[:, ic, 0:401],
                        start=(ic == 0),
                        stop=(ic == 3),
                    )
                rts = pout.tile([1, 1], F32, tag="rts", bufs=4)
                nc.vector.reciprocal(rts[:], ps_ts[0:1, 400:401])
                tsn = pout.tile([1, D], BF16, tag="tsn", bufs=3)
                nc.scalar.mul(tsn[:], ps_ts[0:1, 0:D], rts[:])
                # target_source as per-partition columns (d-major) via 4 tiny
                # K=1 matmuls: psum[q, kc] = tsn[0, kc*128+q]
                tsc_ps = psml_ps.tile([128, 4], F32, tag="tscps", bufs=1, name="tsc_ps")
                for kc in range(4):
                    nc.tensor.matmul(
                        tsc_ps[0:100, kc : kc + 1],
                        lhsT=tsn[0:1, kc:400:4],
                        rhs=ones[0:1, 0:1],
                        start=True,
                        stop=True,
                    )
                tsnc = pout.tile([128, 4], F32, tag="tsnc", bufs=4)
                nc.vector.tensor_copy(tsnc[:], tsc_ps[:])
                # S*target_source computed d-major on DVE (4x mode) against
                # the resident S^T; host transposes this quarter back.
                sxtsT = pout.tile([128, 4, 512], BF16, tag="sxtsT")
                for kc in range(4):
                    p = KC[kc]
                    nc.vector.tensor_scalar_mul(
                        sxtsT[0:p, kc, :], stt[0:p, kc, :], tsnc[0:p, kc : kc + 1]
                    )
                eng2 = nc.sync if last else nc.gpsimd
                eng2.dma_start(
                    out=out2_h[b].rearrange("(p k) c -> p k c", k=4),
                    in_=sxtsT[0:100, :, :],
                )

            prologue_dma(0)
            wm_scale(0)
            prologue_compute(0)
            for b in range(BL):
                simT_pass(b)
                if b + 1 < BL:
                    prologue_dma(b + 1)
                rest_front(b)
                if b > 0:
                    w_tail(b - 1)
                if b + 1 < BL:
                    prologue_compute(b + 1)
            w_tail(BL - 1)
    return nc


_NC_CACHE: list = []


def _get_program() -> bass.Bass:
    if not _NC_CACHE:
        nc = build_program()
        _split_multi_waits(nc)
        _NC_CACHE.append(nc)
    return _NC_CACHE[0]


def _host_shards(S: np.ndarray, T: np.ndarray, w: np.ndarray):
    """Build per-core input maps (pure layout marshalling, no math)."""
    ws, wt, wm = w[:D], w[D : 2 * D], w[2 * D :]
    bf = ml_dtypes.bfloat16
    wmf = np.zeros((128, 4), np.float32)
    wmf[0:100, :] = wm.reshape(100, 4)  # d = 4p + kc
    # packed constants: [ws | wt | ident | ones]
    sing = np.zeros((128, 1056), dtype=bf)
    sing[:, 0:D] = ws.astype(bf)[None, :]
    sing[:, D : 2 * D] = wt.astype(bf)[None, :]
    sing[:, 800:928] = np.eye(128, dtype=bf)
    sing[:, 928:1056] = 1.0

    def aug_rows(X):  # [bl, L, 400] -> [bl, L, 404] bf16 with col 400 = 1.0
        bl = X.shape[0]
        out = np.zeros((bl, X.shape[1], 404), dtype=bf)
        out[:, :, 0:D] = X.astype(bf)
        out[:, :, D] = 1.0
        return out

    in_maps = []
    for c in range(N_CORES):
        Sb = S[c * BL : (c + 1) * BL]
        Tb = T[c * BL : (c + 1) * BL]
        in_maps.append(
            {
                "srow": aug_rows(Sb),
                "trow": aug_rows(Tb),
                "st": np.ascontiguousarray(Sb.transpose(0, 2, 1)).astype(bf),
                "tt": np.ascontiguousarray(Tb.transpose(0, 2, 1)).astype(bf),
                "sing": sing,
                "wmf": wmf,
            }
        )
    return in_maps


def kernel(source_embedding, target_embedding, w_sim, **run_kwargs):
    S = np.asarray(source_embedding, dtype=np.float32)
    T = np.asarray(target_embedding, dtype=np.float32)
    w = np.asarray(w_sim, dtype=np.float32)
    assert S.shape == (B, LS, D) and T.shape == (B, LT, D) and w.shape == (3 * D,)

    nc = _get_program()
    in_maps = _host_shards(S, T, w)
    res = run_bass_kernel_spmd(nc, in_maps, core_ids=list(range(N_CORES)), **run_kwargs)
    dev = np.concatenate(
        [res.results[c]["out"] for c in range(N_CORES)], axis=0
    ).astype(np.float32)
    dev2 = np.concatenate(
        [res.results[c]["out2"] for c in range(N_CORES)], axis=0
    ).astype(np.float32)
    out = np.empty((B, LS, 4 * D), np.float32)
    out[:, :, 0:D] = S  # identity quarter assembled on host
    out[:, :, D : 3 * D] = dev
    out[:, :, 3 * D :] = dev2.transpose(0, 2, 1)  # d-major quarter back to row-major
    if run_kwargs:
        kernel.last_results = res  # expose profile info to test harness
    return out


# revision 67
# speedup vs baseline: 1.0426x; 1.0426x over previous
"""BiDAF attention-flow kernel for 8 Trainium2 NeuronCores (Bass/Tile).

Data-parallel over batch: B=32 -> 4 batches per core on 8 cores.

Math (per batch b):
  sim[i,j] = s_proj[i] + t_proj[j] + sum_d S[i,d]*wm[d]*T[j,d]
  A        = softmax_j(sim)            (row-constant s_proj cancels)
  source_target = A @ T
  w[i]     = exp(max_j sim[i,j]) ; tgt_attn = w / sum(w)
  target_source = tgt_attn @ S         (one row, broadcast over rows)
  out      = [S | source_target | S*source_target | S*target_source]

Device strategy (memory-roofline oriented; ~72us vs 101.5us baseline):
  - The S quarter of the output is the identity; it never touches the device.
    Device emits only the 3 computed quarters, in bf16 (rel-err budget 2e-2).
  - All inputs bf16: S,T in row-major (i on partitions) and d-major (for the
    sim contraction), marshalled on host.  All constants ride in one packed
    [128,1064] DMA (each dma_start costs ~600ns of sequencer issue time).
  - sim^T (j on partitions) is computed ONCE on PE; exp applied on ACT with a
    per-partition bias = t_proj[j].  Projections become per-partition columns:
    t_proj = DVE-mul + ACT Identity-accum, s_proj = DVE scalar_tensor_tensor
    accum — no PE matmuls, no affine-row gymnastics.
  - Row max over j:  max_j exp(sim^T) == exp(max_j sim) (monotone), so the
    4 exp strips are pairwise-maxed on DVE, the [128,512] result is PE-
    transposed per 128-col block, and one DVE free-axis max yields w as
    columns.  s_proj (cancelling in A) is folded in afterwards:
    w = w' * exp(s_proj).
  - softmax normalization deferred: ones-column in the row-major inputs gives
    row sums in column 400 of the A@[T|1] / w@[S|1] matmul outputs.
  - S*target_source is computed d-major (tensor_scalar on the resident S^T at
    DVE 4x rate, scalars = target_source columns extracted by K=1 matmuls)
    and stored transposed; the host transposes it back (pure marshalling).
  - Software pipeline: input DMAs issued a batch ahead (sync engine only),
    wm-scale emitted early in DVE order (it gates the next simT), and each
    batch's target_source chain (w_tail) emitted after the NEXT batch's simT
    so its cross-engine latency hides under PE work.  S*source_target on
    Pool, stf scaling on ACT, separate PSUM rings for sim vs A@T tiles.
"""

import sys

import numpy as np
import ml_dtypes

# concourse is importable via the axon sitecustomize path; fall back to /opt.
try:
    import concourse.bass as bass
except ImportError:  # pragma: no cover
    sys.path.insert(0, "/opt/trn_rl_repo")
    import concourse.bass as bass

import concourse.mybir as mybir
import concourse.tile as tile
from concourse.bass import ts
from concourse.bass_utils import run_bass_kernel_spmd


B, LS, LT, D = 32, 512, 512, 400
N_CORES = 8
BL = B // N_CORES  # batches per core
F32 = mybir.dt.float32
BF16 = mybir.dt.bfloat16
EXP = mybir.ActivationFunctionType.Exp
AX = mybir.AxisListType.X
MULT = mybir.AluOpType.mult
ADD = mybir.AluOpType.add

KC = [128, 128, 128, 16]  # contraction chunks over d (400 = 3*128 + 16)


def _split_multi_waits(nc: bass.Bass) -> None:
    """This walrus build encodes at most ONE sync-wait per instruction
    ("Too many sync wait commands" in setupSyncWait).  Tile's wait pass can
    attach several sem-waits to one instruction; hoist the extras onto
    same-engine NoOp carriers immediately before it (the NX sequencer
    executes the waits in order, so semantics are identical)."""
    ctr = 0
    for fn in nc.m.functions:
        for bb in fn.blocks:
            if not any(
                i.sync_info is not None and len(i.sync_info.on_wait) > 1
                for i in bb.instructions
            ):
                continue
            new_insts = []
            for inst in bb.instructions:
                si = inst.sync_info
                if si is not None and len(si.on_wait) > 1:
                    waits = list(si.on_wait)
                    for w in waits[:-1]:
                        ctr += 1
                        nop = mybir.InstNoOp(
                            name=f"splitw-{ctr}",
                            engine=inst.engine,
                            sync_info=mybir.SyncInfo(on_wait=[w], on_update=[]),
                            bass_nofuse=True,
                        )
                        nc.register_instruction(nop, overwrite=True)
                        new_insts.append(nop)
                    del si.on_wait[:-1]
                new_insts.append(inst)
            bb.instructions[:] = new_insts


def build_program() -> bass.Bass:
    nc = bass.Bass("TRN2", target_bir_lowering=False, debug=False)

    # Per-core DRAM I/O (host feeds per-core shards).
    srow_h = nc.dram_tensor("srow", [BL, LS, 404], BF16, kind="ExternalInput").ap()
    trow_h = nc.dram_tensor("trow", [BL, LT, 404], BF16, kind="ExternalInput").ap()
    st_h = nc.dram_tensor("st", [BL, D, LS], BF16, kind="ExternalInput").ap()
    tt_h = nc.dram_tensor("tt", [BL, D, LT], BF16, kind="ExternalInput").ap()
    # wm first in its own tiny DMA — it gates batch 0's wm-scale
    wmf_h = nc.dram_tensor("wmf", [128, 4], F32, kind="ExternalInput").ap()
    # packed constants: [ws(400) | wt(400) | ident(128) | ones(128)]
    sing_h = nc.dram_tensor("sing", [128, 1056], BF16, kind="ExternalInput").ap()
    out_h = nc.dram_tensor("out", [BL, LS, 800], BF16, kind="ExternalOutput").ap()
    # S*target_source quarter, stored d-major (host re-transposes)
    out2_h = nc.dram_tensor("out2", [BL, D, LS], BF16, kind="ExternalOutput").ap()

    with tile.TileContext(nc) as tc:
        with (
            tc.tile_pool(name="singles", bufs=1) as singles,
            tc.tile_pool(name="pin", bufs=4) as pin,
            tc.tile_pool(name="pet", bufs=2) as pet,
            tc.tile_pool(name="pout", bufs=2) as pout,
            tc.tile_pool(name="pps", bufs=3, space="PSUM") as pps,
            tc.tile_pool(name="ppo", bufs=2, space="PSUM") as ppo,
            tc.tile_pool(name="pmax_ps", bufs=1, space="PSUM") as pmax_ps,
            tc.tile_pool(name="psml_ps", bufs=2, space="PSUM") as psml_ps,
        ):
            wmf = singles.tile([128, 4], F32)
            nc.sync.dma_start(out=wmf[:], in_=wmf_h)
            sing = singles.tile([128, 1056], BF16)
            nc.sync.dma_start(out=sing[:], in_=sing_h)
            wsb = sing[:, 0:400]
            wtb = sing[:, 400:800]
            ident = sing[:, 800:928]
            ones = sing[0:1, 928:1056]

            state = {}

            def prologue_dma(b):
                """Issue batch b input DMAs (sync engine only, chunk-split so
                downstream per-chunk consumers start as soon as data lands).
                Order: ttt/stt first (they gate wm-scale + simT on PE)."""
                srow = pin.tile([128, 4, 404], BF16, tag="srow")
                trow = pin.tile([128, 4, 404], BF16, tag="trow")
                stt = pin.tile([128, 4, 512], BF16, tag="stt")
                ttt = pin.tile([128, 4, 512], BF16, tag="ttt")
                nc.sync.dma_start(out=ttt[:, 0, :], in_=tt_h[b, 0:128, :])
                nc.sync.dma_start(out=stt[:, 0, :], in_=st_h[b, 0:128, :])
                nc.sync.dma_start(
                    out=ttt[:, 1:3, :],
                    in_=tt_h[b, 128:384, :].rearrange("(k p) c -> p k c", p=128),
                )
                nc.sync.dma_start(
                    out=stt[:, 1:3, :],
                    in_=st_h[b, 128:384, :].rearrange("(k p) c -> p k c", p=128),
                )
                nc.sync.dma_start(out=ttt[0:16, 3, :], in_=tt_h[b, 384:400, :])
                nc.sync.dma_start(out=stt[0:16, 3, :], in_=st_h[b, 384:400, :])
                nc.sync.dma_start(
                    out=trow[:], in_=trow_h[b].rearrange("(j p) c -> p j c", p=128)
                )
                nc.sync.dma_start(
                    out=srow[:], in_=srow_h[b].rearrange("(i p) c -> p i c", p=128)
                )
                state[b] = [srow, trow, stt, ttt]

            def prologue_compute(b):
                """Batch b DVE/ACT prep: wm-scale (gates PE simT) first, then
                the projection columns."""
                srow, trow, stt, ttt = state[b][:4]
                # projections as per-partition columns on Pool (gpsimd) —
                # keeps DVE free for the max/scale chain
                tp = pout.tile([128, 4], F32, tag="tp", bufs=4)
                sp = pout.tile([128, 4], F32, tag="sp", bufs=4)
                tg = pout.tile([128, 4, D], BF16, tag="tg")
                sg = pout.tile([128, 4, D], BF16, tag="sg")
                for jc in range(4):
                    nc.vector.tensor_mul(tg[:, jc, :], trow[:, jc, 0:D], wtb[:])
                for jc in range(4):
                    nc.scalar.activation(
                        tg[:, jc, :],
                        tg[:, jc, :],
                        mybir.ActivationFunctionType.Identity,
                        accum_out=tp[:, jc : jc + 1],
                    )
                for ic in range(4):
                    nc.vector.scalar_tensor_tensor(
                        out=sg[:, ic, :],
                        in0=srow[:, ic, 0:D],
                        scalar=1.0,
                        in1=wsb[:],
                        op0=MULT,
                        op1=MULT,
                        accum_out=sp[:, ic : ic + 1],
                    )
                state[b] = [srow, trow, stt, ttt, tp, sp]

            def simT_pass(b):
                """E^T = exp(sim^T + t_proj[j]) with t_proj as ACT bias."""
                srow, trow, stt, ttt, tp, sp = state[b]
                et = pet.tile([128, 4, 512], BF16, tag="et")
                state[b].append(et)
                for jc in range(4):
                    ps = pps.tile([128, 512], F32, tag="psbig")
                    for kc in range(4):
                        p = KC[kc]
                        nc.tensor.matmul(
                            ps[:],
                            lhsT=ttt[0:p, kc, ts(jc, 128)],
                            rhs=stt[0:p, kc, :],
                            start=(kc == 0),
                            stop=(kc == 3),
                        )
                    nc.scalar.activation(
                        et[:, jc, :], ps[:], EXP, bias=tp[:, jc : jc + 1]
                    )

            def wm_scale(b):
                """Scale T^T by wm in place (DVE 4x mode).  Emitted right
                after the previous batch's pairmax so it lands early in DVE
                order — it gates the next batch's simT on PE."""
                ttt = state[b][3]
                for kc in range(4):
                    p = KC[kc]
                    nc.vector.tensor_scalar_mul(
                        ttt[0:p, kc, :], ttt[0:p, kc, :], wmf[0:p, kc : kc + 1]
                    )

            def rest_front(b):
                srow, trow, stt, ttt, tp, sp, et = state[b]

                # pairwise max of the 4 exp strips, split DVE/Pool so the two
                # halves run concurrently
                em2 = pout.tile([128, 2, 512], BF16, tag="em2")
                em = pout.tile([128, 512], BF16, tag="em")
                nc.vector.tensor_max(em2[:, 0, :], et[:, 0, :], et[:, 1, :])
                nc.vector.tensor_max(em2[:, 1, :], et[:, 2, :], et[:, 3, :])
                nc.vector.tensor_max(em[:], em2[:, 0, :], em2[:, 1, :])
                if b + 1 < BL:
                    wm_scale(b + 1)
                spe = pout.tile([128, 4], BF16, tag="spe", bufs=4)

                # epilogue matmuls + row-max transposes, interleaved so PE
                # has A@T work while DVE catches up on the max chain.
                po = {}
                # stf and sxst interleaved per row so both quarters leave in
                # ONE DMA with 1600B contiguous dst rows (descriptor-gen is
                # the DMA bottleneck, and descriptors are per contiguous run)
                sto = pout.tile([128, 4, 2 * D], BF16, tag="sto")
                stf = sto[:, :, 0:D]
                sxst = sto[:, :, D : 2 * D]
                rinv = pout.tile([128, 4], F32, tag="rinv", bufs=4)
                wraw = pout.tile([128, 4], BF16, tag="wraw", bufs=4)
                wtile = pout.tile([128, 4], BF16, tag="wtile", bufs=4)

                last = b == BL - 1

                def at_wave(ic):
                    """source_target += A@[T|1] for i-chunk ic (deferred scale)."""
                    po[ic] = ppo.tile(
                        [128, 512], F32, tag="psbig2", name=f"po{ic}"
                    )
                    for jc in range(4):
                        nc.tensor.matmul(
                            po[ic][:, 0:401],
                            lhsT=et[:, jc, ts(ic, 128)],
                            rhs=trow[:, jc, 0:401],
                            start=(jc == 0),
                            stop=(jc == 3),
                        )
                    nc.vector.reciprocal(rinv[:, ic : ic + 1], po[ic][:, 400:401])
                    if ic == 1:
                        # free po[1]'s PSUM bank early (po[3] reuses it): DVE
                        # scales it right after the reciprocal instead of
                        # queueing behind ACT's exp/stf backlog
                        nc.vector.tensor_scalar_mul(
                            stf[:, ic, :], po[ic][:, 0:D], rinv[:, ic : ic + 1]
                        )
                    else:
                        nc.scalar.mul(
                            stf[:, ic, :], po[ic][:, 0:D], rinv[:, ic : ic + 1]
                        )
                    nc.gpsimd.tensor_mul(
                        sxst[:, ic, :], srow[:, ic, 0:D], stf[:, ic, :]
                    )
                    if last:
                        nc.scalar.dma_start(
                            out=out_h[b, ts(ic, 128), :], in_=sto[:, ic, :]
                        )

                def max_chain():
                    # row-max via PE transpose of em blocks + one merged DVE max
                    tps = pmax_ps.tile([128, 4, 128], BF16, tag="maxps")
                    for ic in range(4):
                        nc.tensor.transpose(
                            tps[:, ic, :], em[:, ts(ic, 128)], ident[:]
                        )
                    for ic in range(4):
                        nc.vector.reduce_max(
                            wraw[:, ic : ic + 1], tps[:, ic, :], axis=AX
                        )
                    # exp(s_proj) for the w fixup — emitted here so the stf
                    # chain ahead of it on ACT isn't delayed
                    nc.scalar.activation(spe[:], sp[:], EXP)
                    nc.vector.tensor_mul(wtile[:], wraw[:], spe[:])

                at_wave(0)
                at_wave(1)
                max_chain()
                at_wave(2)
                at_wave(3)
                if not last:
                    # one DMA for both quarters: 1600B contiguous dst rows
                    nc.gpsimd.dma_start(
                        out=out_h[b].rearrange("(i p) c -> p i c", p=128),
                        in_=sto[:],
                    )
                state[b].append(wtile)

            def w_tail(b):
                """The target_source chain for batch b.  Emitted after batch
                b+1's simT so its cross-engine latency hides under PE work."""
                srow, trow, stt, ttt, tp, sp, et, wtile = state[b]
                last = b == BL - 1

                # target_source = (w @ [S|1]) / sum(w)
                ps_ts = psml_ps.tile([1, 402], F32, tag="pssml", bufs=1, name="ps_ts")
                for ic in range(4):
                    nc.tensor.matmul(
                        ps_ts[0:1, 0:401],
                        lhsT=wtile[:, ic : ic + 1],
                        rhs=srow[:, ic, 0:401],
                        start=(ic == 0),
                        stop=(ic == 3),
                    )
                rts = pout.tile([1, 1], F32, tag="rts", bufs=4)
                nc.vector.reciprocal(rts[:], ps_ts[0:1, 400:401])
                tsn = pout.tile([1, D], BF16, tag="tsn", bufs=3)
                nc.scalar.mul(tsn[:], ps_ts[0:1, 0:D], rts[:])
                # target_source as per-partition columns (d-major) via 4 tiny
                # K=1 matmuls: psum[q, kc] = tsn[0, kc*128+q]
                tsc_ps = psml_ps.tile([128, 4], F32, tag="tscps", bufs=1, name="tsc_ps")
                for kc in range(4):
                    p = KC[kc]
                    nc.tensor.matmul(
                        tsc_ps[0:p, kc : kc + 1],
                        lhsT=tsn[0:1, kc * 128 : kc * 128 + p],
                        rhs=ones[0:1, 0:1],
                        start=True,
                        stop=True,
                    )
                tsnc = pout.tile([128, 4], F32, tag="tsnc", bufs=4)
                nc.vector.tensor_copy(tsnc[:], tsc_ps[:])
                # S*target_source computed d-major on DVE (4x mode) against
                # the resident S^T; host transposes this quarter back.
                sxtsT = pout.tile([128, 4, 512], BF16, tag="sxtsT")
                for kc in range(4):
                    p = KC[kc]
                    nc.vector.tensor_scalar_mul(
                        sxtsT[0:p, kc, :], stt[0:p, kc, :], tsnc[0:p, kc : kc + 1]
                    )
                if last:
                    for kc in range(3):
                        nc.sync.dma_start(
                            out=out2_h[b, ts(kc, 128), :], in_=sxtsT[:, kc, :]
                        )
                else:
                    nc.gpsimd.dma_start(
                        out=out2_h[b, 0:384, :].rearrange("(k p) c -> p k c", p=128),
                        in_=sxtsT[:, 0:3, :],
                    )
                eng2 = nc.sync if last else nc.gpsimd
                eng2.dma_start(out=out2_h[b, 384:400, :], in_=sxtsT[0:16, 3, :])

            prologue_dma(0)
            wm_scale(0)
            prologue_compute(0)
            for b in range(BL):
                simT_pass(b)
                if b + 1 < BL:
                    prologue_dma(b + 1)
                rest_front(b)
                if b > 0:
                    w_tail(b - 1)
                if b + 1 < BL:
                    prologue_compute(b + 1)
            w_tail(BL - 1)
    return nc


_NC_CACHE: list = []


def _get_program() -> bass.Bass:
    if not _NC_CACHE:
        nc = build_program()
        _split_multi_waits(nc)
        _NC_CACHE.append(nc)
    return _NC_CACHE[0]


def _host_shards(S: np.ndarray, T: np.ndarray, w: np.ndarray):
    """Build per-core input maps (pure layout marshalling, no math)."""
    ws, wt, wm = w[:D], w[D : 2 * D], w[2 * D :]
    bf = ml_dtypes.bfloat16
    wmf = np.zeros((128, 4), np.float32)
    for kc in range(4):
        p = KC[kc]
        wmf[0:p, kc] = wm[kc * 128 : kc * 128 + p]
    # packed constants: [ws | wt | ident | ones]
    sing = np.zeros((128, 1056), dtype=bf)
    sing[:, 0:D] = ws.astype(bf)[None, :]
    sing[:, D : 2 * D] = wt.astype(bf)[None, :]
    sing[:, 800:928] = np.eye(128, dtype=bf)
    sing[:, 928:1056] = 1.0

    def aug_rows(X):  # [bl, L, 400] -> [bl, L, 404] bf16 with col 400 = 1.0
        bl = X.shape[0]
        out = np.zeros((bl, X.shape[1], 404), dtype=bf)
        out[:, :, 0:D] = X.astype(bf)
        out[:, :, D] = 1.0
        return out

    in_maps = []
    for c in range(N_CORES):
        Sb = S[c * BL : (c + 1) * BL]
        Tb = T[c * BL : (c + 1) * BL]
        in_maps.append(
            {
                "srow": aug_rows(Sb),
                "trow": aug_rows(Tb),
                "st": np.ascontiguousarray(Sb.transpose(0, 2, 1)).astype(bf),
                "tt": np.ascontiguousarray(Tb.transpose(0, 2, 1)).astype(bf),
                "sing": sing,
                "wmf": wmf,
            }
        )
    return in_maps


def kernel(source_embedding, target_embedding, w_sim, **run_kwargs):
    S = np.asarray(source_embedding, dtype=np.float32)
    T = np.asarray(target_embedding, dtype=np.float32)
    w = np.asarray(w_sim, dtype=np.float32)
    assert S.shape == (B, LS, D) and T.shape == (B, LT, D) and w.shape == (3 * D,)

    nc = _get_program()
    in_maps = _host_shards(S, T, w)
    res = run_bass_kernel_spmd(nc, in_maps, core_ids=list(range(N_CORES)), **run_kwargs)
    dev = np.concatenate(
        [res.results[c]["out"] for c in range(N_CORES)], axis=0
    ).astype(np.float32)
    dev2 = np.concatenate(
        [res.results[c]["out2"] for c in range(N_CORES)], axis=0
    ).astype(np.float32)
    out = np.empty((B, LS, 4 * D), np.float32)
    out[:, :, 0:D] = S  # identity quarter assembled on host
    out[:, :, D : 3 * D] = dev
    out[:, :, 3 * D :] = dev2.transpose(0, 2, 1)  # d-major quarter back to row-major
    if run_kwargs:
        kernel.last_results = res  # expose profile info to test harness
    return out


# revision 68
# speedup vs baseline: 1.0812x; 1.0370x over previous
"""BiDAF attention-flow kernel for 8 Trainium2 NeuronCores (Bass/Tile).

Data-parallel over batch: B=32 -> 4 batches per core on 8 cores.

Math (per batch b):
  sim[i,j] = s_proj[i] + t_proj[j] + sum_d S[i,d]*wm[d]*T[j,d]
  A        = softmax_j(sim)            (row-constant s_proj cancels)
  source_target = A @ T
  w[i]     = exp(max_j sim[i,j]) ; tgt_attn = w / sum(w)
  target_source = tgt_attn @ S         (one row, broadcast over rows)
  out      = [S | source_target | S*source_target | S*target_source]

Device strategy (memory-roofline oriented; ~72us vs 101.5us baseline):
  - The S quarter of the output is the identity; it never touches the device.
    Device emits only the 3 computed quarters, in bf16 (rel-err budget 2e-2).
  - All inputs bf16: S,T in row-major (i on partitions) and d-major (for the
    sim contraction), marshalled on host.  All constants ride in one packed
    [128,1064] DMA (each dma_start costs ~600ns of sequencer issue time).
  - sim^T (j on partitions) is computed ONCE on PE; exp applied on ACT with a
    per-partition bias = t_proj[j].  Projections become per-partition columns:
    t_proj = DVE-mul + ACT Identity-accum, s_proj = DVE scalar_tensor_tensor
    accum — no PE matmuls, no affine-row gymnastics.
  - Row max over j:  max_j exp(sim^T) == exp(max_j sim) (monotone), so the
    4 exp strips are pairwise-maxed on DVE, the [128,512] result is PE-
    transposed per 128-col block, and one DVE free-axis max yields w as
    columns.  s_proj (cancelling in A) is folded in afterwards:
    w = w' * exp(s_proj).
  - softmax normalization deferred: ones-column in the row-major inputs gives
    row sums in column 400 of the A@[T|1] / w@[S|1] matmul outputs.
  - S*target_source is computed d-major (tensor_scalar on the resident S^T at
    DVE 4x rate, scalars = target_source columns extracted by K=1 matmuls)
    and stored transposed; the host transposes it back (pure marshalling).
  - Software pipeline: input DMAs issued a batch ahead (sync engine only),
    wm-scale emitted early in DVE order (it gates the next simT), and each
    batch's target_source chain (w_tail) emitted after the NEXT batch's simT
    so its cross-engine latency hides under PE work.  S*source_target on
    Pool, stf scaling on ACT, separate PSUM rings for sim vs A@T tiles.
"""

import sys

import numpy as np
import ml_dtypes

# concourse is importable via the axon sitecustomize path; fall back to /opt.
try:
    import concourse.bass as bass
except ImportError:  # pragma: no cover
    sys.path.insert(0, "/opt/trn_rl_repo")
    import concourse.bass as bass

import concourse.mybir as mybir
import concourse.tile as tile
from concourse.bass import ts
from concourse.bass_utils import run_bass_kernel_spmd


B, LS, LT, D = 32, 512, 512, 400
N_CORES = 8
BL = B // N_CORES  # batches per core
F32 = mybir.dt.float32
BF16 = mybir.dt.bfloat16
EXP = mybir.ActivationFunctionType.Exp
AX = mybir.AxisListType.X
MULT = mybir.AluOpType.mult
ADD = mybir.AluOpType.add

# d-axis layout: d = 4p + kc with D zero-padded to 512 on host — every
# partition holds 4 consecutive d rows, so each d-major input is ONE DMA of
# 4KB contiguous runs across all 128 partitions (even queue spread;
# descriptor-generation is the DMA bottleneck).  Zero rows add nothing to
# the contraction and matmul cost depends only on the moving dim.
KC = [128, 128, 128, 128]


def _split_multi_waits(nc: bass.Bass) -> None:
    """This walrus build encodes at most ONE sync-wait per instruction
    ("Too many sync wait commands" in setupSyncWait).  Tile's wait pass can
    attach several sem-waits to one instruction; hoist the extras onto
    same-engine NoOp carriers immediately before it (the NX sequencer
    executes the waits in order, so semantics are identical)."""
    ctr = 0
    for fn in nc.m.functions:
        for bb in fn.blocks:
            if not any(
                i.sync_info is not None and len(i.sync_info.on_wait) > 1
                for i in bb.instructions
            ):
                continue
            new_insts = []
            for inst in bb.instructions:
                si = inst.sync_info
                if si is not None and len(si.on_wait) > 1:
                    waits = list(si.on_wait)
                    for w in waits[:-1]:
                        ctr += 1
                        nop = mybir.InstNoOp(
                            name=f"splitw-{ctr}",
                            engine=inst.engine,
                            sync_info=mybir.SyncInfo(on_wait=[w], on_update=[]),
                            bass_nofuse=True,
                        )
                        nc.register_instruction(nop, overwrite=True)
                        new_insts.append(nop)
                    del si.on_wait[:-1]
                new_insts.append(inst)
            bb.instructions[:] = new_insts


def build_program() -> bass.Bass:
    nc = bass.Bass("TRN2", target_bir_lowering=False, debug=False)

    # Per-core DRAM I/O (host feeds per-core shards).
    srow_h = nc.dram_tensor("srow", [BL, LS, 404], BF16, kind="ExternalInput").ap()
    trow_h = nc.dram_tensor("trow", [BL, LT, 404], BF16, kind="ExternalInput").ap()
    st_h = nc.dram_tensor("st", [BL, 512, LS], BF16, kind="ExternalInput").ap()
    tt_h = nc.dram_tensor("tt", [BL, 512, LT], BF16, kind="ExternalInput").ap()
    # wm first in its own tiny DMA — it gates batch 0's wm-scale
    wmf_h = nc.dram_tensor("wmf", [128, 4], F32, kind="ExternalInput").ap()
    # packed constants: [ws(400) | wt(400) | ident(128) | ones(128)]
    sing_h = nc.dram_tensor("sing", [128, 1056], BF16, kind="ExternalInput").ap()
    out_h = nc.dram_tensor("out", [BL, LS, 800], BF16, kind="ExternalOutput").ap()
    # S*target_source quarter, stored d-major (host re-transposes)
    out2_h = nc.dram_tensor("out2", [BL, D, LS], BF16, kind="ExternalOutput").ap()

    with tile.TileContext(nc) as tc:
        with (
            tc.tile_pool(name="singles", bufs=1) as singles,
            tc.tile_pool(name="pin", bufs=4) as pin,
            tc.tile_pool(name="pet", bufs=2) as pet,
            tc.tile_pool(name="pout", bufs=2) as pout,
            tc.tile_pool(name="pps", bufs=3, space="PSUM") as pps,
            tc.tile_pool(name="ppo", bufs=2, space="PSUM") as ppo,
            tc.tile_pool(name="pmax_ps", bufs=1, space="PSUM") as pmax_ps,
            tc.tile_pool(name="psml_ps", bufs=2, space="PSUM") as psml_ps,
        ):
            wmf = singles.tile([128, 4], F32)
            nc.sync.dma_start(out=wmf[:], in_=wmf_h)
            sing = singles.tile([128, 1056], BF16)
            nc.sync.dma_start(out=sing[:], in_=sing_h)
            wsb = sing[:, 0:400]
            wtb = sing[:, 400:800]
            ident = sing[:, 800:928]
            ones = sing[0:1, 928:1056]

            state = {}

            def prologue_dma(b):
                """Issue batch b input DMAs (sync engine only, chunk-split so
                downstream per-chunk consumers start as soon as data lands).
                Order: ttt/stt first (they gate wm-scale + simT on PE)."""
                srow = pin.tile([128, 4, 404], BF16, tag="srow")
                trow = pin.tile([128, 4, 404], BF16, tag="trow")
                stt = pin.tile([128, 4, 512], BF16, tag="stt")
                ttt = pin.tile([128, 4, 512], BF16, tag="ttt")
                nc.sync.dma_start(
                    out=ttt[:], in_=tt_h[b].rearrange("(p k) c -> p k c", k=4)
                )
                nc.sync.dma_start(
                    out=stt[:], in_=st_h[b].rearrange("(p k) c -> p k c", k=4)
                )
                nc.sync.dma_start(
                    out=trow[:], in_=trow_h[b].rearrange("(j p) c -> p j c", p=128)
                )
                nc.sync.dma_start(
                    out=srow[:], in_=srow_h[b].rearrange("(i p) c -> p i c", p=128)
                )
                state[b] = [srow, trow, stt, ttt]

            def prologue_compute(b):
                """Batch b DVE/ACT prep: wm-scale (gates PE simT) first, then
                the projection columns."""
                srow, trow, stt, ttt = state[b][:4]
                # projections as per-partition columns on Pool (gpsimd) —
                # keeps DVE free for the max/scale chain
                tp = pout.tile([128, 4], F32, tag="tp", bufs=4)
                sp = pout.tile([128, 4], F32, tag="sp", bufs=4)
                tg = pout.tile([128, 4, D], BF16, tag="tg")
                sg = pout.tile([128, 4, D], BF16, tag="sg")
                for jc in range(4):
                    nc.vector.tensor_mul(tg[:, jc, :], trow[:, jc, 0:D], wtb[:])
                for jc in range(4):
                    nc.scalar.activation(
                        tg[:, jc, :],
                        tg[:, jc, :],
                        mybir.ActivationFunctionType.Identity,
                        accum_out=tp[:, jc : jc + 1],
                    )
                for ic in range(4):
                    nc.vector.scalar_tensor_tensor(
                        out=sg[:, ic, :],
                        in0=srow[:, ic, 0:D],
                        scalar=1.0,
                        in1=wsb[:],
                        op0=MULT,
                        op1=MULT,
                        accum_out=sp[:, ic : ic + 1],
                    )
                state[b] = [srow, trow, stt, ttt, tp, sp]

            def simT_pass(b):
                """E^T = exp(sim^T + t_proj[j]) with t_proj as ACT bias."""
                srow, trow, stt, ttt, tp, sp = state[b]
                et = pet.tile([128, 4, 512], BF16, tag="et")
                state[b].append(et)
                for jc in range(4):
                    ps = pps.tile([128, 512], F32, tag="psbig")
                    for kc in range(4):
                        p = KC[kc]
                        nc.tensor.matmul(
                            ps[:],
                            lhsT=ttt[0:p, kc, ts(jc, 128)],
                            rhs=stt[0:p, kc, :],
                            start=(kc == 0),
                            stop=(kc == 3),
                        )
                    nc.scalar.activation(
                        et[:, jc, :], ps[:], EXP, bias=tp[:, jc : jc + 1]
                    )

            def wm_scale(b):
                """Scale T^T by wm in place (DVE 4x mode).  Emitted right
                after the previous batch's pairmax so it lands early in DVE
                order — it gates the next batch's simT on PE."""
                ttt = state[b][3]
                for kc in range(4):
                    p = KC[kc]
                    nc.vector.tensor_scalar_mul(
                        ttt[0:p, kc, :], ttt[0:p, kc, :], wmf[0:p, kc : kc + 1]
                    )

            def rest_front(b):
                srow, trow, stt, ttt, tp, sp, et = state[b]

                # pairwise max of the 4 exp strips, split DVE/Pool so the two
                # halves run concurrently
                em2 = pout.tile([128, 2, 512], BF16, tag="em2")
                em = pout.tile([128, 512], BF16, tag="em")
                nc.vector.tensor_max(em2[:, 0, :], et[:, 0, :], et[:, 1, :])
                nc.vector.tensor_max(em2[:, 1, :], et[:, 2, :], et[:, 3, :])
                nc.vector.tensor_max(em[:], em2[:, 0, :], em2[:, 1, :])
                if b + 1 < BL:
                    wm_scale(b + 1)
                spe = pout.tile([128, 4], BF16, tag="spe", bufs=4)

                # epilogue matmuls + row-max transposes, interleaved so PE
                # has A@T work while DVE catches up on the max chain.
                po = {}
                # stf and sxst interleaved per row so both quarters leave in
                # ONE DMA with 1600B contiguous dst rows (descriptor-gen is
                # the DMA bottleneck, and descriptors are per contiguous run)
                sto = pout.tile([128, 4, 2 * D], BF16, tag="sto")
                stf = sto[:, :, 0:D]
                sxst = sto[:, :, D : 2 * D]
                rinv = pout.tile([128, 4], F32, tag="rinv", bufs=4)
                wraw = pout.tile([128, 4], BF16, tag="wraw", bufs=4)
                wtile = pout.tile([128, 4], BF16, tag="wtile", bufs=4)

                last = b == BL - 1

                def at_wave(ic):
                    """source_target += A@[T|1] for i-chunk ic (deferred scale)."""
                    po[ic] = ppo.tile(
                        [128, 512], F32, tag="psbig2", name=f"po{ic}"
                    )
                    for jc in range(4):
                        nc.tensor.matmul(
                            po[ic][:, 0:401],
                            lhsT=et[:, jc, ts(ic, 128)],
                            rhs=trow[:, jc, 0:401],
                            start=(jc == 0),
                            stop=(jc == 3),
                        )
                    nc.vector.reciprocal(rinv[:, ic : ic + 1], po[ic][:, 400:401])
                    if ic == 1:
                        # free po[1]'s PSUM bank early (po[3] reuses it): DVE
                        # scales it right after the reciprocal instead of
                        # queueing behind ACT's exp/stf backlog
                        nc.vector.tensor_scalar_mul(
                            stf[:, ic, :], po[ic][:, 0:D], rinv[:, ic : ic + 1]
                        )
                    else:
                        nc.scalar.mul(
                            stf[:, ic, :], po[ic][:, 0:D], rinv[:, ic : ic + 1]
                        )
                    nc.gpsimd.tensor_mul(
                        sxst[:, ic, :], srow[:, ic, 0:D], stf[:, ic, :]
                    )
                    if last:
                        nc.scalar.dma_start(
                            out=out_h[b, ts(ic, 128), :], in_=sto[:, ic, :]
                        )

                def max_chain():
                    # row-max via PE transpose of em blocks + one merged DVE max
                    tps = pmax_ps.tile([128, 4, 128], BF16, tag="maxps")
                    for ic in range(4):
                        nc.tensor.transpose(
                            tps[:, ic, :], em[:, ts(ic, 128)], ident[:]
                        )
                    for ic in range(4):
                        nc.vector.reduce_max(
                            wraw[:, ic : ic + 1], tps[:, ic, :], axis=AX
                        )
                    # exp(s_proj) for the w fixup — emitted here so the stf
                    # chain ahead of it on ACT isn't delayed
                    nc.scalar.activation(spe[:], sp[:], EXP)
                    nc.vector.tensor_mul(wtile[:], wraw[:], spe[:])

                at_wave(0)
                at_wave(1)
                max_chain()
                at_wave(2)
                at_wave(3)
                if not last:
                    # one DMA for both quarters: 1600B contiguous dst rows
                    nc.gpsimd.dma_start(
                        out=out_h[b].rearrange("(i p) c -> p i c", p=128),
                        in_=sto[:],
                    )
                state[b].append(wtile)

            def w_tail(b):
                """The target_source chain for batch b.  Emitted after batch
                b+1's simT so its cross-engine latency hides under PE work."""
                srow, trow, stt, ttt, tp, sp, et, wtile = state[b]
                last = b == BL - 1

                # target_source = (w @ [S|1]) / sum(w)
                ps_ts = psml_ps.tile([1, 402], F32, tag="pssml", bufs=1, name="ps_ts")
                for ic in range(4):
                    nc.tensor.matmul(
                        ps_ts[0:1, 0:401],
                        lhsT=wtile[:, ic : ic + 1],
                        rhs=srow[:, ic, 0:401],
                        start=(ic == 0),
                        stop=(ic == 3),
                    )
                rts = pout.tile([1, 1], F32, tag="rts", bufs=4)
                nc.vector.reciprocal(rts[:], ps_ts[0:1, 400:401])
                tsn = pout.tile([1, D], BF16, tag="tsn", bufs=3)
                nc.scalar.mul(tsn[:], ps_ts[0:1, 0:D], rts[:])
                # target_source as per-partition columns (d-major) via 4 tiny
                # K=1 matmuls: psum[q, kc] = tsn[0, kc*128+q]
                tsc_ps = psml_ps.tile([128, 4], F32, tag="tscps", bufs=1, name="tsc_ps")
                for kc in range(4):
                    nc.tensor.matmul(
                        tsc_ps[0:100, kc : kc + 1],
                        lhsT=tsn[0:1, kc:400:4],
                        rhs=ones[0:1, 0:1],
                        start=True,
                        stop=True,
                    )
                tsnc = pout.tile([128, 4], F32, tag="tsnc", bufs=4)
                nc.vector.tensor_copy(tsnc[:], tsc_ps[:])
                # S*target_source computed d-major on DVE (4x mode) against
                # the resident S^T; host transposes this quarter back.
                sxtsT = pout.tile([128, 4, 512], BF16, tag="sxtsT")
                nc.vector.tensor_scalar_mul(
                    sxtsT[0:100, :, :], stt[0:100, :, :], tsnc[0:100, 0:1]
                ) if False else None
                for kc in range(4):
                    nc.vector.tensor_scalar_mul(
                        sxtsT[0:100, kc, :], stt[0:100, kc, :], tsnc[0:100, kc : kc + 1]
                    )
                eng2 = nc.sync if last else nc.gpsimd
                eng2.dma_start(
                    out=out2_h[b].rearrange("(p k) c -> p k c", k=4),
                    in_=sxtsT[0:100, :, :],
                )

            prologue_dma(0)
            wm_scale(0)
            prologue_compute(0)
            for b in range(BL):
                simT_pass(b)
                if b + 1 < BL:
                    prologue_dma(b + 1)
                rest_front(b)
                if b > 0:
                    w_tail(b - 1)
                if b + 1 < BL:
                    prologue_compute(b + 1)
            w_tail(BL - 1)
    return nc


_NC_CACHE: list = []


def _get_program() -> bass.Bass:
    if not _NC_CACHE:
        nc = build_program()
        _split_multi_waits(nc)
        _NC_CACHE.append(nc)
    return _NC_CACHE[0]


def _host_shards(S: np.ndarray, T: np.ndarray, w: np.ndarray):
    """Build per-core input maps (pure layout marshalling, no math)."""
    ws, wt, wm = w[:D], w[D : 2 * D], w[2 * D :]
    bf = ml_dtypes.bfloat16
    wmf = np.zeros((128, 4), np.float32)
    wmf[0:100, :] = wm.reshape(100, 4)  # d = 4p + kc
    # packed constants: [ws | wt | ident | ones]
    sing = np.zeros((128, 1056), dtype=bf)
    sing[:, 0:D] = ws.astype(bf)[None, :]
    sing[:, D : 2 * D] = wt.astype(bf)[None, :]
    sing[:, 800:928] = np.eye(128, dtype=bf)
    sing[:, 928:1056] = 1.0

    def aug_rows(X):  # [bl, L, 400] -> [bl, L, 404] bf16 with col 400 = 1.0
        bl = X.shape[0]
        out = np.zeros((bl, X.shape[1], 404), dtype=bf)
        out[:, :, 0:D] = X.astype(bf)
        out[:, :, D] = 1.0
        return out

    def pad_t(X):  # [bl, L, 400] -> d-major zero-padded to 512 rows
        out = np.zeros((BL, 512, LS), dtype=bf)
        out[:, 0:D, :] = X.transpose(0, 2, 1).astype(bf)
        return out

    in_maps = []
    for c in range(N_CORES):
        Sb = S[c * BL : (c + 1) * BL]
        Tb = T[c * BL : (c + 1) * BL]
        in_maps.append(
            {
                "srow": aug_rows(Sb),
                "trow": aug_rows(Tb),
                "st": pad_t(Sb),
                "tt": pad_t(Tb),
                "sing": sing,
                "wmf": wmf,
            }
        )
    return in_maps


def kernel(source_embedding, target_embedding, w_sim, **run_kwargs):
    S = np.asarray(source_embedding, dtype=np.float32)
    T = np.asarray(target_embedding, dtype=np.float32)
    w = np.asarray(w_sim, dtype=np.float32)
    assert S.shape == (B, LS, D) and T.shape == (B, LT, D) and w.shape == (3 * D,)

    nc = _get_program()
    in_maps = _host_shards(S, T, w)
    res = run_bass_kernel_spmd(nc, in_maps, core_ids=list(range(N_CORES)), **run_kwargs)
    dev = np.concatenate(
        [res.results[c]["out"] for c in range(N_CORES)], axis=0
    ).astype(np.float32)
    dev2 = np.concatenate(
        [res.results[c]["out2"] for c in range(N_CORES)], axis=0
    ).astype(np.float32)
    out = np.empty((B, LS, 4 * D), np.float32)
    out[:, :, 0:D] = S  # identity quarter assembled on host
    out[:, :, D : 3 * D] = dev
    out[:, :, 3 * D :] = dev2.transpose(0, 2, 1)  # d-major quarter back to row-major
    if run_kwargs:
        kernel.last_results = res  # expose profile info to test harness
    return out


# revision 69
# speedup vs baseline: 1.2326x; 1.1400x over previous
"""BiDAF attention-flow kernel for 8 Trainium2 NeuronCores (Bass/Tile).

Data-parallel over batch: B=32 -> 4 batches per core on 8 cores.

Math (per batch b):
  sim[i,j] = s_proj[i] + t_proj[j] + sum_d S[i,d]*wm[d]*T[j,d]
  A        = softmax_j(sim)            (row-constant s_proj cancels)
  source_target = A @ T
  w[i]     = exp(max_j sim[i,j]) ; tgt_attn = w / sum(w)
  target_source = tgt_attn @ S         (one row, broadcast over rows)
  out      = [S | source_target | S*source_target | S*target_source]

Device strategy (memory-roofline oriented; ~66us vs 101.5us baseline):
  - The S quarter of the output is the identity; it never touches the device.
    Device emits only the 3 computed quarters, in bf16 (rel-err budget 2e-2).
  - All inputs bf16: S,T in row-major (i on partitions) and d-major (for the
    sim contraction), marshalled on host.  All constants ride in one packed
    DMA (each dma_start costs ~600ns of sequencer issue time).
  - DMA is descriptor-generation-bound, not bandwidth-bound, so layouts
    maximize contiguous-run length: the d-major tensors use d = 4p + kc with
    D zero-padded to 512 (one DMA each, 4KB runs, all 128 partitions so the
    partition-assigned queues load evenly), and the two row-major output
    quarters interleave in one SBUF tile (one DMA, 1600B runs).
  - sim^T (j on partitions) is computed ONCE on PE; exp applied on ACT with a
    per-partition bias = t_proj[j].  Projections become per-partition columns:
    t_proj = DVE-mul + ACT Identity-accum, s_proj = DVE scalar_tensor_tensor
    accum — no PE matmuls, no affine-row gymnastics.
  - Row max over j:  max_j exp(sim^T) == exp(max_j sim) (monotone), so the
    4 exp strips are pairwise-maxed on DVE, the [128,512] result is PE-
    transposed per 128-col block, and one DVE free-axis max yields w as
    columns.  s_proj (cancelling in A) is folded in afterwards:
    w = w' * exp(s_proj).
  - softmax normalization deferred: ones-column in the row-major inputs gives
    row sums in column 400 of the A@[T|1] / w@[S|1] matmul outputs.
  - S*target_source is computed d-major (tensor_scalar on the resident S^T at
    DVE 4x rate, scalars = target_source columns extracted by K=1 matmuls)
    and stored transposed; the host transposes it back (pure marshalling).
  - Software pipeline: input DMAs issued a batch ahead (sync engine only),
    wm-scale emitted early in DVE order (it gates the next simT), and each
    batch's target_source chain (w_tail) emitted after the NEXT batch's simT
    so its cross-engine latency hides under PE work.  S*source_target on
    Pool, stf scaling on ACT, separate PSUM rings for sim vs A@T tiles.
"""

import sys

import numpy as np
import ml_dtypes

# concourse is importable via the axon sitecustomize path; fall back to /opt.
try:
    import concourse.bass as bass
except ImportError:  # pragma: no cover
    sys.path.insert(0, "/opt/trn_rl_repo")
    import concourse.bass as bass

import concourse.mybir as mybir
import concourse.tile as tile
from concourse.bass import ts
from concourse.bass_utils import run_bass_kernel_spmd


B, LS, LT, D = 32, 512, 512, 400
N_CORES = 8
BL = B // N_CORES  # batches per core
F32 = mybir.dt.float32
BF16 = mybir.dt.bfloat16
EXP = mybir.ActivationFunctionType.Exp
AX = mybir.AxisListType.X
MULT = mybir.AluOpType.mult
ADD = mybir.AluOpType.add

# d-axis layout: d = 4p + kc with D zero-padded to 512 on host — every
# partition holds 4 consecutive d rows, so each d-major input is ONE DMA of
# 4KB contiguous runs across all 128 partitions (even queue spread;
# descriptor-generation is the DMA bottleneck).  Zero rows add nothing to
# the contraction and matmul cost depends only on the moving dim.
KC = [128, 128, 128, 128]


def _split_multi_waits(nc: bass.Bass) -> None:
    """This walrus build encodes at most ONE sync-wait per instruction
    ("Too many sync wait commands" in setupSyncWait).  Tile's wait pass can
    attach several sem-waits to one instruction; hoist the extras onto
    same-engine NoOp carriers immediately before it (the NX sequencer
    executes the waits in order, so semantics are identical)."""
    ctr = 0
    for fn in nc.m.functions:
        for bb in fn.blocks:
            if not any(
                i.sync_info is not None and len(i.sync_info.on_wait) > 1
                for i in bb.instructions
            ):
                continue
            new_insts = []
            for inst in bb.instructions:
                si = inst.sync_info
                if si is not None and len(si.on_wait) > 1:
                    waits = list(si.on_wait)
                    for w in waits[:-1]:
                        ctr += 1
                        nop = mybir.InstNoOp(
                            name=f"splitw-{ctr}",
                            engine=inst.engine,
                            sync_info=mybir.SyncInfo(on_wait=[w], on_update=[]),
                            bass_nofuse=True,
                        )
                        nc.register_instruction(nop, overwrite=True)
                        new_insts.append(nop)
                    del si.on_wait[:-1]
                new_insts.append(inst)
            bb.instructions[:] = new_insts


def build_program() -> bass.Bass:
    nc = bass.Bass("TRN2", target_bir_lowering=False, debug=False)

    # Per-core DRAM I/O (host feeds per-core shards).
    srow_h = nc.dram_tensor("srow", [BL, LS, 404], BF16, kind="ExternalInput").ap()
    trow_h = nc.dram_tensor("trow", [BL, LT, 404], BF16, kind="ExternalInput").ap()
    st_h = nc.dram_tensor("st", [BL, 512, LS], BF16, kind="ExternalInput").ap()
    tt_h = nc.dram_tensor("tt", [BL, 512, LT], BF16, kind="ExternalInput").ap()
    # wm first in its own tiny DMA — it gates batch 0's wm-scale
    wmf_h = nc.dram_tensor("wmf", [128, 4], F32, kind="ExternalInput").ap()
    # packed constants: [ws(400) | wt(400) | ident(128) | ones(128)]
    sing_h = nc.dram_tensor("sing", [128, 1056], BF16, kind="ExternalInput").ap()
    out_h = nc.dram_tensor("out", [BL, LS, 800], BF16, kind="ExternalOutput").ap()
    # S*target_source quarter, stored d-major (host re-transposes)
    out2_h = nc.dram_tensor("out2", [BL, D, LS], BF16, kind="ExternalOutput").ap()

    with tile.TileContext(nc) as tc:
        with (
            tc.tile_pool(name="singles", bufs=1) as singles,
            tc.tile_pool(name="pin", bufs=4) as pin,
            tc.tile_pool(name="pet", bufs=2) as pet,
            tc.tile_pool(name="pout", bufs=2) as pout,
            tc.tile_pool(name="pps", bufs=3, space="PSUM") as pps,
            tc.tile_pool(name="ppo", bufs=2, space="PSUM") as ppo,
            tc.tile_pool(name="pmax_ps", bufs=1, space="PSUM") as pmax_ps,
            tc.tile_pool(name="psml_ps", bufs=2, space="PSUM") as psml_ps,
        ):
            wmf = singles.tile([128, 4], F32)
            nc.sync.dma_start(out=wmf[:], in_=wmf_h)
            sing = singles.tile([128, 1056], BF16)
            nc.sync.dma_start(out=sing[:], in_=sing_h)
            wsb = sing[:, 0:400]
            wtb = sing[:, 400:800]
            ident = sing[:, 800:928]
            ones = sing[0:1, 928:1056]

            state = {}

            def prologue_dma(b):
                """Issue batch b input DMAs (sync engine only, chunk-split so
                downstream per-chunk consumers start as soon as data lands).
                Order: ttt/stt first (they gate wm-scale + simT on PE)."""
                srow = pin.tile([128, 4, 404], BF16, tag="srow")
                trow = pin.tile([128, 4, 404], BF16, tag="trow")
                stt = pin.tile([128, 4, 512], BF16, tag="stt")
                ttt = pin.tile([128, 4, 512], BF16, tag="ttt")
                nc.sync.dma_start(
                    out=ttt[:], in_=tt_h[b].rearrange("(p k) c -> p k c", k=4)
                )
                nc.sync.dma_start(
                    out=stt[:], in_=st_h[b].rearrange("(p k) c -> p k c", k=4)
                )
                nc.sync.dma_start(
                    out=trow[:], in_=trow_h[b].rearrange("(j p) c -> p j c", p=128)
                )
                nc.sync.dma_start(
                    out=srow[:], in_=srow_h[b].rearrange("(i p) c -> p i c", p=128)
                )
                state[b] = [srow, trow, stt, ttt]

            def prologue_compute(b):
                """Batch b DVE/ACT prep: wm-scale (gates PE simT) first, then
                the projection columns."""
                srow, trow, stt, ttt = state[b][:4]
                # projections as per-partition columns on Pool (gpsimd) —
                # keeps DVE free for the max/scale chain
                tp = pout.tile([128, 4], F32, tag="tp", bufs=4)
                sp = pout.tile([128, 4], F32, tag="sp", bufs=4)
                tg = pout.tile([128, 4, D], BF16, tag="tg")
                sg = pout.tile([128, 4, D], BF16, tag="sg")
                for jc in range(4):
                    nc.vector.tensor_mul(tg[:, jc, :], trow[:, jc, 0:D], wtb[:])
                for jc in range(4):
                    nc.scalar.activation(
                        tg[:, jc, :],
                        tg[:, jc, :],
                        mybir.ActivationFunctionType.Identity,
                        accum_out=tp[:, jc : jc + 1],
                    )
                for ic in range(4):
                    nc.vector.scalar_tensor_tensor(
                        out=sg[:, ic, :],
                        in0=srow[:, ic, 0:D],
                        scalar=1.0,
                        in1=wsb[:],
                        op0=MULT,
                        op1=MULT,
                        accum_out=sp[:, ic : ic + 1],
                    )
                state[b] = [srow, trow, stt, ttt, tp, sp]

            def simT_pass(b):
                """E^T = exp(sim^T + t_proj[j]) with t_proj as ACT bias."""
                srow, trow, stt, ttt, tp, sp = state[b]
                et = pet.tile([128, 4, 512], BF16, tag="et")
                state[b].append(et)
                for jc in range(4):
                    ps = pps.tile([128, 512], F32, tag="psbig")
                    for kc in range(4):
                        p = KC[kc]
                        nc.tensor.matmul(
                            ps[:],
                            lhsT=ttt[0:p, kc, ts(jc, 128)],
                            rhs=stt[0:p, kc, :],
                            start=(kc == 0),
                            stop=(kc == 3),
                        )
                    nc.scalar.activation(
                        et[:, jc, :], ps[:], EXP, bias=tp[:, jc : jc + 1]
                    )

            def wm_scale(b):
                """Scale T^T by wm in place (DVE 4x mode).  Emitted right
                after the previous batch's pairmax so it lands early in DVE
                order — it gates the next batch's simT on PE."""
                ttt = state[b][3]
                for kc in range(4):
                    p = KC[kc]
                    nc.vector.tensor_scalar_mul(
                        ttt[0:p, kc, :], ttt[0:p, kc, :], wmf[0:p, kc : kc + 1]
                    )

            def rest_front(b):
                srow, trow, stt, ttt, tp, sp, et = state[b]

                # pairwise max of the 4 exp strips, split DVE/Pool so the two
                # halves run concurrently
                em2 = pout.tile([128, 2, 512], BF16, tag="em2")
                em = pout.tile([128, 512], BF16, tag="em")
                nc.vector.tensor_max(em2[:, 0, :], et[:, 0, :], et[:, 1, :])
                nc.vector.tensor_max(em2[:, 1, :], et[:, 2, :], et[:, 3, :])
                nc.vector.tensor_max(em[:], em2[:, 0, :], em2[:, 1, :])
                if b + 1 < BL:
                    wm_scale(b + 1)
                spe = pout.tile([128, 4], BF16, tag="spe", bufs=4)

                # epilogue matmuls + row-max transposes, interleaved so PE
                # has A@T work while DVE catches up on the max chain.
                po = {}
                # stf and sxst interleaved per row so both quarters leave in
                # ONE DMA with 1600B contiguous dst rows (descriptor-gen is
                # the DMA bottleneck, and descriptors are per contiguous run)
                sto = pout.tile([128, 4, 2 * D], BF16, tag="sto")
                stf = sto[:, :, 0:D]
                sxst = sto[:, :, D : 2 * D]
                rinv = pout.tile([128, 4], F32, tag="rinv", bufs=4)
                wraw = pout.tile([128, 4], BF16, tag="wraw", bufs=4)
                wtile = pout.tile([128, 4], BF16, tag="wtile", bufs=4)

                last = b == BL - 1

                def at_wave(ic):
                    """source_target += A@[T|1] for i-chunk ic (deferred scale)."""
                    po[ic] = ppo.tile(
                        [128, 512], F32, tag="psbig2", name=f"po{ic}"
                    )
                    for jc in range(4):
                        nc.tensor.matmul(
                            po[ic][:, 0:401],
                            lhsT=et[:, jc, ts(ic, 128)],
                            rhs=trow[:, jc, 0:401],
                            start=(jc == 0),
                            stop=(jc == 3),
                        )
                    nc.vector.reciprocal(rinv[:, ic : ic + 1], po[ic][:, 400:401])
                    if ic == 1:
                        # free po[1]'s PSUM bank early (po[3] reuses it): DVE
                        # scales it right after the reciprocal instead of
                        # queueing behind ACT's exp/stf backlog
                        nc.vector.tensor_scalar_mul(
                            stf[:, ic, :], po[ic][:, 0:D], rinv[:, ic : ic + 1]
                        )
                    else:
                        nc.scalar.mul(
                            stf[:, ic, :], po[ic][:, 0:D], rinv[:, ic : ic + 1]
                        )
                    nc.gpsimd.tensor_mul(
                        sxst[:, ic, :], srow[:, ic, 0:D], stf[:, ic, :]
                    )
                    if last:
                        nc.scalar.dma_start(
                            out=out_h[b, ts(ic, 128), :], in_=sto[:, ic, :]
                        )

                def max_chain():
                    # row-max via PE transpose of em blocks + one merged DVE max
                    tps = pmax_ps.tile([128, 4, 128], BF16, tag="maxps")
                    for ic in range(4):
                        nc.tensor.transpose(
                            tps[:, ic, :], em[:, ts(ic, 128)], ident[:]
                        )
                    for ic in range(4):
                        nc.vector.reduce_max(
                            wraw[:, ic : ic + 1], tps[:, ic, :], axis=AX
                        )
                    # exp(s_proj) for the w fixup — emitted here so the stf
                    # chain ahead of it on ACT isn't delayed
                    nc.scalar.activation(spe[:], sp[:], EXP)
                    nc.vector.tensor_mul(wtile[:], wraw[:], spe[:])

                at_wave(0)
                at_wave(1)
                max_chain()
                at_wave(2)
                at_wave(3)
                if not last:
                    # one DMA for both quarters: 1600B contiguous dst rows
                    nc.gpsimd.dma_start(
                        out=out_h[b].rearrange("(i p) c -> p i c", p=128),
                        in_=sto[:],
                    )
                state[b].append(wtile)

            def w_tail(b):
                """The target_source chain for batch b.  Emitted after batch
                b+1's simT so its cross-engine latency hides under PE work."""
                srow, trow, stt, ttt, tp, sp, et, wtile = state[b]
                last = b == BL - 1

                # target_source = (w @ [S|1]) / sum(w)
                ps_ts = psml_ps.tile([1, 402], F32, tag="pssml", bufs=1, name="ps_ts")
                for ic in range(4):
                    nc.tensor.matmul(
                        ps_ts[0:1, 0:401],
                        lhsT=wtile[:, ic : ic + 1],
                        rhs=srow[:, ic, 0:401],
                        start=(ic == 0),
                        stop=(ic == 3),
                    )
                rts = pout.tile([1, 1], F32, tag="rts", bufs=4)
                nc.vector.reciprocal(rts[:], ps_ts[0:1, 400:401])
                tsn = pout.tile([1, D], BF16, tag="tsn", bufs=3)
                nc.scalar.mul(tsn[:], ps_ts[0:1, 0:D], rts[:])
                # target_source as per-partition columns (d-major) via 4 tiny
                # K=1 matmuls: psum[q, kc] = tsn[0, kc*128+q]
                tsc_ps = psml_ps.tile([128, 4], F32, tag="tscps", bufs=1, name="tsc_ps")
                for kc in range(4):
                    nc.tensor.matmul(
                        tsc_ps[0:100, kc : kc + 1],
                        lhsT=tsn[0:1, kc:400:4],
                        rhs=ones[0:1, 0:1],
                        start=True,
                        stop=True,
                    )
                tsnc = pout.tile([128, 4], F32, tag="tsnc", bufs=4)
                nc.vector.tensor_copy(tsnc[:], tsc_ps[:])
                # S*target_source computed d-major on DVE (4x mode) against
                # the resident S^T; host transposes this quarter back.
                sxtsT = pout.tile([128, 4, 512], BF16, tag="sxtsT")
                nc.vector.tensor_scalar_mul(
                    sxtsT[0:100, :, :], stt[0:100, :, :], tsnc[0:100, 0:1]
                ) if False else None
                for kc in range(4):
                    nc.vector.tensor_scalar_mul(
                        sxtsT[0:100, kc, :], stt[0:100, kc, :], tsnc[0:100, kc : kc + 1]
                    )
                eng2 = nc.sync if last else nc.gpsimd
                eng2.dma_start(
                    out=out2_h[b].rearrange("(p k) c -> p k c", k=4),
                    in_=sxtsT[0:100, :, :],
                )

            prologue_dma(0)
            wm_scale(0)
            prologue_compute(0)
            for b in range(BL):
                simT_pass(b)
                if b + 1 < BL:
                    prologue_dma(b + 1)
                rest_front(b)
                if b > 0:
                    w_tail(b - 1)
                if b + 1 < BL:
                    prologue_compute(b + 1)
            w_tail(BL - 1)
    return nc


_NC_CACHE: list = []


def _get_program() -> bass.Bass:
    if not _NC_CACHE:
        nc = build_program()
        _split_multi_waits(nc)
        _NC_CACHE.append(nc)
    return _NC_CACHE[0]


def _host_shards(S: np.ndarray, T: np.ndarray, w: np.ndarray):
    """Build per-core input maps (pure layout marshalling, no math)."""
    ws, wt, wm = w[:D], w[D : 2 * D], w[2 * D :]
    bf = ml_dtypes.bfloat16
    wmf = np.zeros((128, 4), np.float32)
    wmf[0:100, :] = wm.reshape(100, 4)  # d = 4p + kc
    # packed constants: [ws | wt | ident | ones]
    sing = np.zeros((128, 1056), dtype=bf)
    sing[:, 0:D] = ws.astype(bf)[None, :]
    sing[:, D : 2 * D] = wt.astype(bf)[None, :]
    sing[:, 800:928] = np.eye(128, dtype=bf)
    sing[:, 928:1056] = 1.0

    def aug_rows(X):  # [bl, L, 400] -> [bl, L, 404] bf16 with col 400 = 1.0
        bl = X.shape[0]
        out = np.zeros((bl, X.shape[1], 404), dtype=bf)
        out[:, :, 0:D] = X.astype(bf)
        out[:, :, D] = 1.0
        return out

    def pad_t(X):  # [bl, L, 400] -> d-major zero-padded to 512 rows
        out = np.zeros((BL, 512, LS), dtype=bf)
        out[:, 0:D, :] = X.transpose(0, 2, 1).astype(bf)
        return out

    in_maps = []
    for c in range(N_CORES):
        Sb = S[c * BL : (c + 1) * BL]
        Tb = T[c * BL : (c + 1) * BL]
        in_maps.append(
            {
                "srow": aug_rows(Sb),
                "trow": aug_rows(Tb),
                "st": pad_t(Sb),
                "tt": pad_t(Tb),
                "sing": sing,
                "wmf": wmf,
            }
        )
    return in_maps


def kernel(source_embedding, target_embedding, w_sim, **run_kwargs):
    S = np.asarray(source_embedding, dtype=np.float32)
    T = np.asarray(target_embedding, dtype=np.float32)
    w = np.asarray(w_sim, dtype=np.float32)
    assert S.shape == (B, LS, D) and T.shape == (B, LT, D) and w.shape == (3 * D,)

    nc = _get_program()
    in_maps = _host_shards(S, T, w)
    res = run_bass_kernel_spmd(nc, in_maps, core_ids=list(range(N_CORES)), **run_kwargs)
    dev = np.concatenate(
        [res.results[c]["out"] for c in range(N_CORES)], axis=0
    ).astype(np.float32)
    dev2 = np.concatenate(
        [res.results[c]["out2"] for c in range(N_CORES)], axis=0
    ).astype(np.float32)
    out = np.empty((B, LS, 4 * D), np.float32)
    out[:, :, 0:D] = S  # identity quarter assembled on host
    out[:, :, D : 3 * D] = dev
    out[:, :, 3 * D :] = dev2.transpose(0, 2, 1)  # d-major quarter back to row-major
    if run_kwargs:
        kernel.last_results = res  # expose profile info to test harness
    return out
